# revision 30
# baseline (speedup 1.0000x reference)
"""Trainium2 Bass kernel for nn_HFMiMoV2DecoderLayer (attention + MoE decoder layer).

Strategy (8 NeuronCores):
  Host    — rmsnorm1 folded: xnt = (x * rsqrt(mean x^2))^T uploaded once.
  Launch 1 — tensor-parallel attention: each core owns 2 of 16 heads (and the
    matching GQA KV head). QKV computed weights-stationary producing Q^T/K^T
    directly in the [head-dim, token] layout phase B consumes (no x or QK
    transposes on chip). Rope runs in the transposed layout using a signed
    half-swap matmul. Flash-style causal sink-softmax with 512-wide query
    groups, software-pipelined with both heads interleaved; normalization
    folded into the O^T PSUM evacuation; the Wo product for each query group
    is emitted right after the group completes so the fp32 partial write
    overlaps later groups' compute. The pre-gate path must stay near-fp32:
    min routing margin for this layer's data is ~5.6e-5.
  Host    — h1 = x + sum(partials); exact MoE routing (numpy, mirrors the
    reference); builds per-expert gathered activation matrices (bf16).
  Launch 2 — expert-parallel MoE FF in bf16 (post-gate, tolerance allows it):
    each core owns 2 of 16 experts; chunk-outer loop overlaps down-projection
    with gate/up; combine weights folded into the down-proj PSUM evacuation
    as a per-partition scalar scale; contributions returned bf16.
  Host    — scatter-add contributions into h1.
"""
import sys
import types

import ml_dtypes
import numpy as np

BF16_NP = ml_dtypes.bfloat16


def _install_ntff_hook():
    """bass_utils needs antenv.axon_hooks for NTFF tracing under axon; the
    image's antenv lacks that submodule. Inject a shim wired to the ctypes
    hook from trn_agent_boot (no-op if anything is missing)."""
    if "antenv.axon_hooks" in sys.modules:
        return
    try:
        from trn_agent_boot.trn_boot import _ntff_profile_via_ctypes

        hook = _ntff_profile_via_ctypes("/opt/axon/libaxon_pjrt.so")
    except Exception:
        hook = None
    mod = types.ModuleType("antenv.axon_hooks")
    mod._hook = hook
    mod.set_axon_ntff_profile_hook = lambda h: setattr(mod, "_hook", h)
    mod.get_axon_ntff_profile_hook = lambda: mod._hook
    sys.modules["antenv.axon_hooks"] = mod


_install_ntff_hook()

import concourse.bass as bass
import concourse.mybir as mybir
import concourse.tile as tile
from concourse import bacc
from concourse.bass_utils import run_bass_kernel_spmd
from concourse.masks import make_identity

F32 = mybir.dt.float32
F32R = mybir.dt.float32r
BF16 = mybir.dt.bfloat16

N_CORES = 8
T = 2048          # tokens
H = 2048          # hidden
P = 128
TCH = T // P      # 16 token chunks
HCH = H // P      # 16 hidden chunks
HD = 128          # head dim
NHC = 2           # heads per core
RD = 64           # rope dims
RH = 32
FF = 512          # moe intermediate
E = 16
EPC = 2           # experts per core
SCALE = HD ** -0.5
EPS = 1e-6
ROUTE_SCALE = 2.5
G, TG, TK = 4, 2, 4

QG = 512          # query-group width for attention phase B
NQG = T // QG     # 4
ACH = 512         # token-chunk width for phase A
NACH = T // ACH   # 4
LAG = 2           # phase B software-pipeline depth (steps)


def _r32(ap):
    return ap.bitcast(F32R)


def _mk_nc():
    return bacc.Bacc("TRN2", target_bir_lowering=False, debug=False,
                     num_devices=N_CORES)


# --------------------------------------------------------------------------
# Launch 1: attention (2 heads per core)
# --------------------------------------------------------------------------

def build_attn():
    nc = _mk_nc()
    # xnt = (rmsnorm(x))^T  [H, T]
    xnt = nc.dram_tensor("xnt", [H, T], F32R, kind="ExternalInput")
    # w_all = [Wq_h0 | Wq_h1 | Wk | Wv] with ln1 folded  [H, 512]
    w_all = nc.dram_tensor("w_all", [H, 4 * P], F32R, kind="ExternalInput")
    wo = nc.dram_tensor("wo", [NHC * HD, H], F32R, kind="ExternalInput")
    # rope tables in transposed layout [64, T]
    cost = nc.dram_tensor("cost", [RD, T], F32, kind="ExternalInput")
    sint = nc.dram_tensor("sint", [RD, T], F32, kind="ExternalInput")
    # signed half-swap for rotate_half in transposed layout [64, 64]
    srot = nc.dram_tensor("srot", [RD, RD], F32R, kind="ExternalInput")
    sinke = nc.dram_tensor("sinke", [1, NHC], F32, kind="ExternalInput")
    # triangular causal mask for the 128-wide diagonal sub-block
    masks = nc.dram_tensor("masks", [P, P], F32R, kind="ExternalInput")
    partial = nc.dram_tensor("partial", [T, H], F32, kind="ExternalOutput")

    pt_out = partial.rearrange("(tc p) h -> tc p h", p=P)
    w_r = w_all.rearrange("(hc p) n -> hc p n", p=P)
    x_r = xnt.rearrange("(hc p) t -> hc p t", p=P)
    wo_r = wo.rearrange("(h p) n -> h p n", p=P)

    with tile.TileContext(nc) as tc:
        with (
            tc.tile_pool(name="persist", bufs=1) as pers,
            tc.tile_pool(name="const", bufs=1) as constp,
        ):
            w_s = pers.tile([P, HCH, 4 * P], F32R)
            cos_s = pers.tile([RD, T], F32)
            sin_s = pers.tile([RD, T], F32)
            srot_s = pers.tile([RD, RD], F32R)
            mask_s = pers.tile([P, P], F32R)
            sinke_s = pers.tile([1, NHC], F32)

            ident0 = constp.tile([P, P], F32)
            make_identity(nc, ident0[:])
            ident = constp.tile([P, P], F32R)
            nc.vector.tensor_copy(ident[:], ident0[:])
            ones0 = constp.tile([P, P], F32)
            nc.vector.memset(ones0[:], 1.0)
            ones_row = constp.tile([1, P], F32R)
            nc.vector.tensor_copy(ones_row[:], ones0[0:1, :])
            ones_col = constp.tile([P, 1], F32R)
            nc.vector.tensor_copy(ones_col[:], ones0[:, 0:1])

            qkt_s = pers.tile([P, 3, T], F32R)       # [hd, {q0,q1,k}, tok]
            v_s = pers.tile([P, TCH, HD], F32R)      # [tok, tc, hd]
            ot_s = pers.tile([P, NHC, T], F32R)      # O^T [hd, h, tok]
            wo_s = pers.tile([P, NHC, H], F32R)      # staged during phase A

            # ---------------- phase A: QKV^T + rope ----------------
            with (
                tc.tile_pool(name="xin", bufs=2) as xpool,
                tc.tile_pool(name="vtmp", bufs=2) as vtp,
                tc.tile_pool(name="rope", bufs=3) as ropep,
                tc.tile_pool(name="psA", bufs=3, space="PSUM") as psA,
                tc.tile_pool(name="psR", bufs=2, space="PSUM") as psR,
                tc.tile_pool(name="psV", bufs=2, space="PSUM") as psV,
            ):
                # staging: interleave w/x per hidden-chunk so compute starts
                # immediately; rope tables follow, masks (phase B) last
                xnt_first = xpool.tile([P, HCH, ACH], F32R, tag="xnt")
                for hc in range(HCH):
                    nc.sync.dma_start(w_s[:, hc, :], w_r[hc])
                    nc.sync.dma_start(xnt_first[:, hc, :], x_r[hc][:, 0:ACH])
                nc.sync.dma_start(cos_s[:], cost[:])
                nc.sync.dma_start(sin_s[:], sint[:])
                nc.sync.dma_start(srot_s[:], srot[:])
                nc.sync.dma_start(sinke_s[:], sinke[:])
                nc.sync.dma_start(mask_s[:], masks[:])

                # QKV matmuls first as one uninterrupted PE stream; the
                # V-transposes and rope (which each bounce PE->vector->PE)
                # are batched afterwards so their stalls don't break the
                # stream and reset the p-state ramp.
                vt_tiles = []
                for i in range(NACH):
                    t0 = i * ACH
                    if i == 0:
                        xnt_i = xnt_first
                    else:
                        xnt_i = xpool.tile([P, HCH, ACH], F32R, tag="xnt")
                        for hc in range(HCH):
                            nc.sync.dma_start(xnt_i[:, hc, :],
                                              x_r[hc][:, t0:t0 + ACH])
                    # 4 col-blocks: q0, q1, k, v
                    for cb in range(4):
                        ps_c = psA.tile([P, ACH], F32, tag="psc")
                        for hc in range(HCH):
                            nc.tensor.matmul(
                                ps_c[:], _r32(w_s[:, hc, cb * P:(cb + 1) * P]),
                                _r32(xnt_i[:, hc, :]),
                                start=(hc == 0), stop=(hc == HCH - 1))
                        if cb < 3:
                            nc.vector.tensor_copy(qkt_s[:, cb, t0:t0 + ACH],
                                                  ps_c[:])
                        else:
                            vt_sb = vtp.tile([P, ACH], F32R, tag=f"vt{i}",
                                             name=f"vt_sb{i}")
                            nc.vector.tensor_copy(vt_sb[:], ps_c[:])
                            vt_tiles.append(vt_sb)
                # stage Wo during phase B (first needed by phase C of group 0)
                for h in range(NHC):
                    nc.sync.dma_start(wo_s[:, h, :], wo_r[h])
                # V^T -> V [tok, hd]
                for i in range(NACH):
                    for j in range(ACH // P):
                        ps_v = psV.tile([P, P], F32R, tag="psv")
                        nc.tensor.transpose(
                            ps_v[:], vt_tiles[i][:, j * P:(j + 1) * P],
                            ident[:])
                        nc.vector.tensor_copy(
                            v_s[:, i * (ACH // P) + j, :], ps_v[:])
                # rope on rows 0:RD of q0, q1, k
                for i in range(NACH):
                    t0 = i * ACH
                    for b in range(3):
                        q_ap = qkt_s[0:RD, b, t0:t0 + ACH]
                        ps_r = psR.tile([RD, ACH], F32, tag="psr")
                        nc.tensor.matmul(ps_r[:], _r32(srot_s[:]), _r32(q_ap),
                                         start=True, stop=True)
                        t1 = ropep.tile([RD, ACH], F32, tag="t1")
                        nc.gpsimd.tensor_mul(t1[:], q_ap, cos_s[:, t0:t0 + ACH])
                        t2 = ropep.tile([RD, ACH], F32, tag="t2")
                        nc.vector.tensor_mul(t2[:], ps_r[:],
                                             sin_s[:, t0:t0 + ACH])
                        nc.gpsimd.tensor_add(q_ap, t1[:], t2[:])

            # ---------------- phase B + C fused ----------------
            # One psum tag ("ps") is shared by scores, Wo-product, and the
            # reciprocal broadcast so all four buffers stay busy.  Each query
            # group's Wo matmuls ("phase C") are injected into the NEXT
            # group's score/PV step loop so the PE always has independent
            # work while the scalar engine chews through exp.
            with (
                tc.tile_pool(name="psX", bufs=4, space="PSUM") as psX,
                tc.tile_pool(name="psO", bufs=1, space="PSUM") as psO,
                tc.tile_pool(name="psD", bufs=1, space="PSUM") as psD,
                tc.tile_pool(name="ptp", bufs=LAG + 2) as ptp,
                tc.tile_pool(name="den", bufs=2) as denp,
                tc.tile_pool(name="outp", bufs=4) as outp,
            ):
                c_work = []   # pending phase-C emitters from the previous qg

                def emit_c(ti, nt):
                    ps_p = psX.tile([P, 512], F32, tag="ps", name="ps_p")
                    for h in range(NHC):
                        nc.tensor.matmul(
                            ps_p[:],
                            _r32(ot_s[:, h, ti * P:(ti + 1) * P]),
                            _r32(wo_s[:, h, nt * 512:(nt + 1) * 512]),
                            start=(h == 0), stop=(h == NHC - 1))
                    out_sb = outp.tile([P, 512], F32, tag="osb")
                    nc.vector.tensor_copy(out_sb[:], ps_p[:])
                    nc.sync.dma_start(
                        pt_out[ti][:, nt * 512:(nt + 1) * 512], out_sb[:])

                for qg in range(NQG):
                    q0 = qg * QG
                    nkt = (QG // P) * (qg + 1)
                    ps_o = [psO.tile([P, QG], F32, tag=f"pso{h}",
                                     name=f"ps_o{h}")
                            for h in range(NHC)]
                    ps_d = [psD.tile([1, QG], F32, tag=f"psd{h}",
                                     name=f"ps_d{h}")
                            for h in range(NHC)]
                    q_rhs = [_r32(qkt_s[:, h, q0:q0 + QG]) for h in range(NHC)]

                    steps = [(kt, h) for kt in range(nkt) for h in range(NHC)]
                    # interleave leftover phase-C tiles ~evenly across steps
                    cper = -(-len(c_work) // len(steps))
                    pend = []

                    def emit_front(kt, h):
                        # diagonal block: columns < d*128 are fully masked —
                        # skip them in scores/exp/PV/den entirely
                        d = kt - (QG // P) * qg
                        c0 = d * P if d > 0 else 0
                        ps_s = psX.tile([P, QG], F32, tag="ps", name="ps_s")
                        nc.tensor.matmul(
                            ps_s[:, c0:], _r32(qkt_s[:, 2, kt * P:(kt + 1) * P]),
                            q_rhs[h][:, c0:], start=True, stop=True)
                        p_t = ptp.tile([P, QG], F32R, tag="pt")
                        nc.scalar.activation(
                            p_t[:, c0:], ps_s[:, c0:],
                            mybir.ActivationFunctionType.Exp, scale=SCALE)
                        if d >= 0:  # triangular mask on the 128-wide diagonal
                            nc.gpsimd.tensor_mul(
                                p_t[:, d * P:(d + 1) * P],
                                p_t[:, d * P:(d + 1) * P], mask_s[:])
                        return c0, p_t

                    def emit_back(kt, h, c0, p_t):
                        nc.tensor.matmul(ps_o[h][:, c0:],
                                         _r32(v_s[:, kt, :]),
                                         _r32(p_t[:, c0:]),
                                         start=(kt == 0),
                                         stop=(kt == nkt - 1))
                        nc.tensor.matmul(ps_d[h][:, c0:],
                                         _r32(ones_col[:]), _r32(p_t[:, c0:]),
                                         start=(kt == 0),
                                         stop=(kt == nkt - 1))

                    for si, (kt, h) in enumerate(steps):
                        pend.append((kt, h, *emit_front(kt, h)))
                        for _ in range(cper):
                            if c_work:
                                emit_c(*c_work.pop(0))
                        if len(pend) > LAG:
                            emit_back(*pend.pop(0))
                    for item in pend:
                        emit_back(*item)
                    while c_work:
                        emit_c(*c_work.pop(0))

                    for h in range(NHC):
                        den = denp.tile([1, QG], F32, tag="den")
                        nc.vector.tensor_scalar(
                            den[:], ps_d[h][:],
                            sinke_s[:, h:h + 1], None,
                            mybir.AluOpType.add)
                        rec = denp.tile([1, QG], F32R, tag="rec")
                        with nc.allow_low_precision(reason="f32r for PE bcast"):
                            nc.vector.reciprocal(rec[:], den[:])
                        ps_bc = psX.tile([P, QG], F32, tag="ps", name="ps_bc")
                        nc.tensor.matmul(ps_bc[:], _r32(ones_row[:]),
                                         _r32(rec[:]),
                                         start=True, stop=True)
                        rec_bc = denp.tile([P, QG], F32, tag="recbc")
                        nc.scalar.activation(rec_bc[:], ps_bc[:],
                                             mybir.ActivationFunctionType.Copy)
                        nc.vector.tensor_mul(ot_s[:, h, q0:q0 + QG],
                                             ps_o[h][:], rec_bc[:])

                    c_work = [(qg * (QG // P) + tl, nt)
                              for tl in range(QG // P)
                              for nt in range(H // 512)]

                # drain the last query group's phase C
                while c_work:
                    emit_c(*c_work.pop(0))

    nc.finalize()
    return nc


# --------------------------------------------------------------------------
# Launch 2: MoE expert FF (2 experts per core, capacity c_cap tokens each)
# --------------------------------------------------------------------------

def _n_chunks(c):
    """Split c (multiple of 128) into moving-dim chunks of <=512 (bf16)."""
    out = [512] * (c // 512)
    if c % 512:
        out.append(c % 512)
    assert sum(out) == c and all(x % P == 0 for x in out)
    return out


def build_moe(c_cap):
    nc = _mk_nc()
    xgt = nc.dram_tensor("xgt", [EPC, H, c_cap], BF16, kind="ExternalInput")
    # combine weights in token-partition layout: [EPC * c_cap/P, P]
    wcol = nc.dram_tensor("wcol", [EPC * (c_cap // P), P], F32,
                          kind="ExternalInput")
    weg = nc.dram_tensor("weg", [EPC, H, FF], BF16, kind="ExternalInput")
    weu = nc.dram_tensor("weu", [EPC, H, FF], BF16, kind="ExternalInput")
    wed = nc.dram_tensor("wed", [EPC, FF, H], BF16, kind="ExternalInput")
    contrib = nc.dram_tensor("contrib", [EPC * c_cap, H], BF16,
                             kind="ExternalOutput")
    co = contrib.rearrange("(ec tc p) h -> ec tc p h", p=P, ec=EPC)
    wc_r = wcol.rearrange("(ec tc) p -> ec tc p", ec=EPC)
    xg_r = [xgt[e].rearrange("(hc p) c -> hc p c", p=P) for e in range(EPC)]
    wg_r = [weg[e].rearrange("(hc p) f -> hc p f", p=P) for e in range(EPC)]
    wu_r = [weu[e].rearrange("(hc p) f -> hc p f", p=P) for e in range(EPC)]

    nch = _n_chunks(c_cap)
    ffc_n = FF // P  # 4

    with tile.TileContext(nc) as tc:
        with (
            tc.tile_pool(name="xg", bufs=1) as xgp,
            tc.tile_pool(name="wgu", bufs=2) as wgup,
            tc.tile_pool(name="wd", bufs=2) as wdp,
            tc.tile_pool(name="hgu", bufs=2) as hgup,
            tc.tile_pool(name="act", bufs=3) as actp,
            tc.tile_pool(name="wr", bufs=2) as wrp,
            tc.tile_pool(name="outp", bufs=4) as outp,
            tc.tile_pool(name="psG", bufs=2, space="PSUM") as psG,
            tc.tile_pool(name="psU", bufs=2, space="PSUM") as psU,
            tc.tile_pool(name="psC", bufs=3, space="PSUM") as psC,
        ):
            # prefetch everything in compute order: all of expert 0's
            # inputs, then expert 1's (DMA total ~23MB < compute time)
            stage = {}
            for e in range(EPC):
                xg_s = xgp.tile([P, HCH, c_cap], BF16, tag=f"xg{e}",
                                name=f"xg{e}")
                for hc in range(HCH):
                    nc.sync.dma_start(xg_s[:, hc, :], xg_r[e][hc])
                wg_s = wgup.tile([P, HCH, FF], BF16, tag="wg", name=f"wg{e}")
                wu_s = wgup.tile([P, HCH, FF], BF16, tag="wu", name=f"wu{e}")
                for hc in range(HCH):
                    nc.sync.dma_start(wg_s[:, hc, :], wg_r[e][hc])
                    nc.sync.dma_start(wu_s[:, hc, :], wu_r[e][hc])
                wcol_s = wrp.tile([P, c_cap // P], F32, tag="wcol",
                                  name=f"wcol{e}")
                nc.sync.dma_start(wcol_s[:],
                                  wc_r[e].rearrange("tc p -> p tc"))
                wd_s = wdp.tile([P, ffc_n, H], BF16, tag="wd", name=f"wd{e}")
                nc.sync.dma_start(wd_s[:],
                                  wed[e].rearrange("(fc p) h -> p fc h", p=P))
                stage[e] = (xg_s, wg_s, wu_s, wcol_s, wd_s)

            for e in range(EPC):
                xg_s, wg_s, wu_s, wcol_s, wd_s = stage[e]
                nco = 0
                for nsz in nch:
                    hgu = hgup.tile([P, ffc_n, 512], BF16, tag="hgu")
                    for fc in range(ffc_n):
                        ps_g = psG.tile([P, 512], F32, tag="psg")
                        ps_u = psU.tile([P, 512], F32, tag="psu")
                        for hc in range(HCH):
                            nc.tensor.matmul(
                                ps_g[:, :nsz],
                                wg_s[:, hc, fc * P:(fc + 1) * P],
                                xg_s[:, hc, nco:nco + nsz],
                                start=(hc == 0), stop=(hc == HCH - 1))
                            nc.tensor.matmul(
                                ps_u[:, :nsz],
                                wu_s[:, hc, fc * P:(fc + 1) * P],
                                xg_s[:, hc, nco:nco + nsz],
                                start=(hc == 0), stop=(hc == HCH - 1))
                        sg = actp.tile([P, 512], F32, tag="sg")
                        nc.scalar.activation(sg[:, :nsz], ps_g[:, :nsz],
                                             mybir.ActivationFunctionType.Silu)
                        nc.vector.tensor_mul(hgu[:, fc, :nsz],
                                             sg[:, :nsz], ps_u[:, :nsz])
                    # down projection for this chunk's token tiles
                    for tl in range(nsz // P):
                        ti = nco // P + tl
                        for nt in range(H // 512):
                            ps_c = psC.tile([P, 512], F32, tag="psc")
                            for fc in range(ffc_n):
                                nc.tensor.matmul(
                                    ps_c[:],
                                    hgu[:, fc, tl * P:(tl + 1) * P],
                                    wd_s[:, fc, nt * 512:(nt + 1) * 512],
                                    start=(fc == 0), stop=(fc == ffc_n - 1))
                            out_sb = outp.tile([P, 512], BF16, tag="osb")
                            nc.scalar.activation(
                                out_sb[:], ps_c[:],
                                mybir.ActivationFunctionType.Copy,
                                scale=wcol_s[:, ti:ti + 1])
                            nc.sync.dma_start(
                                co[e, ti][:, nt * 512:(nt + 1) * 512],
                                out_sb[:])
                    nco += nsz

    nc.finalize()
    return nc


# --------------------------------------------------------------------------
# Host-side routing (numpy mirror of the reference MoE gate)
# --------------------------------------------------------------------------

def _routing(h1, ln2_w, gate_w, gate_bias):
    var = np.mean(h1 * h1, axis=-1, keepdims=True)
    xf = (ln2_w * (h1 / np.sqrt(var + EPS))).astype(np.float32)
    logits = xf @ gate_w.T
    s = 1.0 / (1.0 + np.exp(-logits))
    sfc = s + gate_bias[None]
    n = sfc.shape[0]
    gview = sfc.reshape(n, G, E // G)
    gsort = np.sort(gview, axis=-1)
    group_scores = gsort[..., -1] + gsort[..., -2]
    gidx = np.argsort(-group_scores, kind="stable", axis=-1)[:, :TG]
    gmask = np.zeros((n, G), np.bool_)
    np.put_along_axis(gmask, gidx, True, axis=1)
    smask = np.repeat(gmask, E // G, axis=1)
    tmp = np.where(smask, sfc, -np.inf)
    tidx = np.argsort(-tmp, kind="stable", axis=-1)[:, :TK]
    tw = np.take_along_axis(s, tidx, axis=1)
    tw = tw / (tw.sum(-1, keepdims=True) + 1e-20)
    tw = tw * ROUTE_SCALE
    cw = np.zeros((n, E), np.float32)
    np.put_along_axis(cw, tidx, tw.astype(np.float32), axis=1)
    return xf, cw


# --------------------------------------------------------------------------
# Entry point
# --------------------------------------------------------------------------

_NC_CACHE = {}


def _get_nc(key, builder, *args):
    if key not in _NC_CACHE:
        _NC_CACHE[key] = builder(*args)
    return _NC_CACHE[key]


def kernel(hidden_states, cos, sin, ln1_w, ln2_w, Wq, Wk, Wv, Wo,
           sink_bias, gate_w, gate_bias, Weg, Weu, Wed, _profile=None):
    hidden_states, cos, sin, ln1_w, ln2_w = map(
        np.asarray, (hidden_states, cos, sin, ln1_w, ln2_w))
    Wq, Wk, Wv, Wo, sink_bias = map(np.asarray, (Wq, Wk, Wv, Wo, sink_bias))
    gate_w, gate_bias, Weg, Weu, Wed = map(
        np.asarray, (gate_w, gate_bias, Weg, Weu, Wed))
    b, s, _ = hidden_states.shape
    x = np.ascontiguousarray(hidden_states.reshape(T, H), dtype=np.float32)

    # host: rmsnorm1 + transpose (pre-gate path stays fp32)
    var = np.mean(x * x, axis=-1, keepdims=True)
    xn = x * (1.0 / np.sqrt(var + EPS))
    xnt = np.ascontiguousarray(xn.T)

    # rope tables in transposed layout
    cosb = cos.reshape(T, RD).astype(np.float32)
    sinb = sin.reshape(T, RD).astype(np.float32)
    cost = np.ascontiguousarray(cosb.T)
    sint = np.ascontiguousarray(sinb.T)

    # signed half-swap: rot[j] = -q[j+RH] (j<RH), q[j-RH] (j>=RH)
    srot = np.zeros((RD, RD), np.float32)
    for j in range(RH):
        srot[j + RH, j] = -1.0
        srot[j, j + RH] = 1.0

    # fold ln1 into the QKV weights
    wq_f = (ln1_w[:, None] * Wq).astype(np.float32)
    wk_f = (ln1_w[:, None] * Wk).astype(np.float32)
    wv_f = (ln1_w[:, None] * Wv).astype(np.float32)

    # triangular causal mask for the 128-wide diagonal sub-block
    kp = np.arange(P)[:, None]
    qf = np.arange(P)[None, :]
    masks = (qf >= kp).astype(np.float32)

    in_maps = []
    for c in range(N_CORES):
        h0 = NHC * c
        g0 = h0 // (16 // 4)  # kv head
        w_cat = np.concatenate(
            [wq_f[:, h0 * HD:(h0 + NHC) * HD],
             wk_f[:, g0 * HD:(g0 + 1) * HD],
             wv_f[:, g0 * HD:(g0 + 1) * HD]], axis=1)
        in_maps.append({
            "xnt": xnt,
            "w_all": np.ascontiguousarray(w_cat),
            "wo": np.ascontiguousarray(Wo[h0 * HD:(h0 + NHC) * HD, :]),
            "cost": cost,
            "sint": sint,
            "srot": srot,
            "sinke": np.exp(sink_bias[h0:h0 + NHC]).reshape(1, NHC)
                       .astype(np.float32),
            "masks": masks,
        })

    nc1 = _get_nc("attn", build_attn)
    res1 = run_bass_kernel_spmd(nc1, in_maps, core_ids=list(range(N_CORES)),
                                trace=_profile is not None)
    h1 = x.copy()
    for c in range(N_CORES):
        h1 += res1.results[c]["partial"]

    xf, cw = _routing(h1, np.asarray(ln2_w), np.asarray(gate_w),
                      np.asarray(gate_bias))

    idxs = [np.nonzero(cw[:, e] > 0)[0] for e in range(E)]
    maxc = max(len(ix) for ix in idxs)
    c_cap = max(P, -(-maxc // P) * P)

    xf_t = xf.T  # [H, n]
    in_maps2 = []
    for c in range(N_CORES):
        xg = np.zeros((EPC, H, c_cap), BF16_NP)
        wr = np.zeros((EPC, c_cap // P, P), np.float32)
        for j in range(EPC):
            e = EPC * c + j
            ix = idxs[e]
            xg[j, :, :len(ix)] = xf_t[:, ix].astype(BF16_NP)
            wr[j].reshape(-1)[:len(ix)] = cw[ix, e]
        in_maps2.append({
            "xgt": xg,
            "wcol": wr.reshape(EPC * (c_cap // P), P),
            "weg": np.ascontiguousarray(Weg[EPC * c:EPC * (c + 1)]
                                        .astype(BF16_NP)),
            "weu": np.ascontiguousarray(Weu[EPC * c:EPC * (c + 1)]
                                        .astype(BF16_NP)),
            "wed": np.ascontiguousarray(Wed[EPC * c:EPC * (c + 1)]
                                        .astype(BF16_NP)),
        })

    nc2 = _get_nc(("moe", c_cap), build_moe, c_cap)
    res2 = run_bass_kernel_spmd(nc2, in_maps2, core_ids=list(range(N_CORES)),
                                trace=_profile is not None)

    out = h1
    for c in range(N_CORES):
        cb = res2.results[c]["contrib"].reshape(EPC, c_cap, H)
        for j in range(EPC):
            e = EPC * c + j
            ix = idxs[e]
            out[ix] += cb[j, :len(ix)].astype(np.float32)

    if _profile is not None:
        _profile["attn_ns"] = res1.exec_time_ns
        _profile["moe_ns"] = res2.exec_time_ns
        _profile["res1"] = res1
        _profile["res2"] = res2

    return out.reshape(b, s, H)


# revision 33
# speedup vs baseline: 1.0418x; 1.0418x over previous
"""Trainium2 Bass kernel for nn_HFMiMoV2DecoderLayer (attention + MoE decoder layer).

Strategy (8 NeuronCores):
  Host    — rmsnorm1 folded: xnt = (x * rsqrt(mean x^2))^T uploaded once.
  Launch 1 — tensor-parallel attention: each core owns 2 of 16 heads (and the
    matching GQA KV head). QKV computed weights-stationary producing Q^T/K^T
    directly in the [head-dim, token] layout phase B consumes (no x or QK
    transposes on chip). Rope runs in the transposed layout using a signed
    half-swap matmul. Flash-style causal sink-softmax with 512-wide query
    groups, software-pipelined with both heads interleaved; normalization
    folded into the O^T PSUM evacuation; the Wo product for each query group
    is emitted right after the group completes so the fp32 partial write
    overlaps later groups' compute. The pre-gate path must stay near-fp32:
    min routing margin for this layer's data is ~5.6e-5.
  Host    — h1 = x + sum(partials); exact MoE routing (numpy, mirrors the
    reference); builds per-expert gathered activation matrices (bf16).
  Launch 2 — expert-parallel MoE FF in bf16 (post-gate, tolerance allows it):
    each core owns 2 of 16 experts; chunk-outer loop overlaps down-projection
    with gate/up; combine weights folded into the down-proj PSUM evacuation
    as a per-partition scalar scale; contributions returned bf16.
  Host    — scatter-add contributions into h1.
"""
import sys
import types

import ml_dtypes
import numpy as np

BF16_NP = ml_dtypes.bfloat16


def _install_ntff_hook():
    """bass_utils needs antenv.axon_hooks for NTFF tracing under axon; the
    image's antenv lacks that submodule. Inject a shim wired to the ctypes
    hook from trn_agent_boot (no-op if anything is missing)."""
    if "antenv.axon_hooks" in sys.modules:
        return
    try:
        from trn_agent_boot.trn_boot import _ntff_profile_via_ctypes

        hook = _ntff_profile_via_ctypes("/opt/axon/libaxon_pjrt.so")
    except Exception:
        hook = None
    mod = types.ModuleType("antenv.axon_hooks")
    mod._hook = hook
    mod.set_axon_ntff_profile_hook = lambda h: setattr(mod, "_hook", h)
    mod.get_axon_ntff_profile_hook = lambda: mod._hook
    sys.modules["antenv.axon_hooks"] = mod


_install_ntff_hook()

import concourse.bass as bass
import concourse.mybir as mybir
import concourse.tile as tile
from concourse import bacc
from concourse.bass_utils import run_bass_kernel_spmd
from concourse.masks import make_identity

F32 = mybir.dt.float32
F32R = mybir.dt.float32r
BF16 = mybir.dt.bfloat16

N_CORES = 8
T = 2048          # tokens
H = 2048          # hidden
P = 128
TCH = T // P      # 16 token chunks
HCH = H // P      # 16 hidden chunks
HD = 128          # head dim
NHC = 2           # heads per core
RD = 64           # rope dims
RH = 32
FF = 512          # moe intermediate
E = 16
EPC = 2           # experts per core
SCALE = HD ** -0.5
EPS = 1e-6
ROUTE_SCALE = 2.5
G, TG, TK = 4, 2, 4

QG = 512          # query-group width for attention phase B
NQG = T // QG     # 4
ACH = 512         # token-chunk width for phase A
NACH = T // ACH   # 4
LAG = 3           # phase B software-pipeline depth (steps)


def _r32(ap):
    return ap.bitcast(F32R)


def _mk_nc():
    return bacc.Bacc("TRN2", target_bir_lowering=False, debug=False,
                     num_devices=N_CORES)


# --------------------------------------------------------------------------
# Launch 1: attention (2 heads per core)
# --------------------------------------------------------------------------

def build_attn():
    nc = _mk_nc()
    # xnt = (rmsnorm(x))^T  [H, T]
    xnt = nc.dram_tensor("xnt", [H, T], F32R, kind="ExternalInput")
    # w_all = [Wq_h0 | Wq_h1 | Wk | Wv] with ln1 folded  [H, 512]
    w_all = nc.dram_tensor("w_all", [H, 4 * P], F32R, kind="ExternalInput")
    wo = nc.dram_tensor("wo", [NHC * HD, H], F32R, kind="ExternalInput")
    # rope tables in transposed layout [64, T]
    cost = nc.dram_tensor("cost", [RD, T], F32, kind="ExternalInput")
    sint = nc.dram_tensor("sint", [RD, T], F32, kind="ExternalInput")
    # signed half-swap for rotate_half in transposed layout [64, 64]
    srot = nc.dram_tensor("srot", [RD, RD], F32R, kind="ExternalInput")
    sinke = nc.dram_tensor("sinke", [1, NHC], F32, kind="ExternalInput")
    # triangular causal mask for the 128-wide diagonal sub-block
    masks = nc.dram_tensor("masks", [P, P], F32R, kind="ExternalInput")
    partial = nc.dram_tensor("partial", [T, H], F32, kind="ExternalOutput")

    pt_out = partial.rearrange("(tc p) h -> tc p h", p=P)
    w_r = w_all.rearrange("(hc p) n -> hc p n", p=P)
    x_r = xnt.rearrange("(hc p) t -> hc p t", p=P)
    wo_r = wo.rearrange("(h p) n -> h p n", p=P)

    with tile.TileContext(nc) as tc:
        with (
            tc.tile_pool(name="persist", bufs=1) as pers,
            tc.tile_pool(name="const", bufs=1) as constp,
        ):
            w_s = pers.tile([P, HCH, 4 * P], F32R)
            cos_s = pers.tile([RD, T], F32)
            sin_s = pers.tile([RD, T], F32)
            srot_s = pers.tile([RD, RD], F32R)
            mask_s = pers.tile([P, P], F32R)
            sinke_s = pers.tile([1, NHC], F32)

            ident0 = constp.tile([P, P], F32)
            make_identity(nc, ident0[:])
            ident = constp.tile([P, P], F32R)
            nc.vector.tensor_copy(ident[:], ident0[:])
            ones0 = constp.tile([P, P], F32)
            nc.vector.memset(ones0[:], 1.0)
            ones_row = constp.tile([1, P], F32R)
            nc.vector.tensor_copy(ones_row[:], ones0[0:1, :])
            ones_col = constp.tile([P, 1], F32R)
            nc.vector.tensor_copy(ones_col[:], ones0[:, 0:1])

            qkt_s = pers.tile([P, 3, T], F32R)       # [hd, {q0,q1,k}, tok]
            v_s = pers.tile([P, TCH, HD], F32R)      # [tok, tc, hd]
            ot_s = pers.tile([P, NHC, T], F32R)      # O^T [hd, h, tok]
            wo_s = pers.tile([P, NHC, H], F32R)      # staged during phase A

            # ---------------- phase A: QKV^T + rope ----------------
            with (
                tc.tile_pool(name="xin", bufs=2) as xpool,
                tc.tile_pool(name="vtmp", bufs=2) as vtp,
                tc.tile_pool(name="rope", bufs=3) as ropep,
                tc.tile_pool(name="psA", bufs=3, space="PSUM") as psA,
                tc.tile_pool(name="psR", bufs=2, space="PSUM") as psR,
                tc.tile_pool(name="psV", bufs=2, space="PSUM") as psV,
            ):
                # staging: interleave w/x per hidden-chunk so compute starts
                # immediately; rope tables follow, masks (phase B) last
                xnt_first = xpool.tile([P, HCH, ACH], F32R, tag="xnt")
                for hc in range(HCH):
                    nc.sync.dma_start(w_s[:, hc, :], w_r[hc])
                    nc.sync.dma_start(xnt_first[:, hc, :], x_r[hc][:, 0:ACH])
                nc.sync.dma_start(cos_s[:], cost[:])
                nc.sync.dma_start(sin_s[:], sint[:])
                nc.sync.dma_start(srot_s[:], srot[:])
                nc.sync.dma_start(sinke_s[:], sinke[:])
                nc.sync.dma_start(mask_s[:], masks[:])

                # QKV matmul stream, DMA-paced.  The V-transposes and rope
                # of chunk i-1 (each a PE->vector->PE bounce) are injected
                # between chunk i's col-blocks to fill the PE's DMA-wait
                # gaps instead of forming a serial batch at the end.
                def mk_vtr(i, vt_sb, j):
                    def emit():
                        ps_v = psV.tile([P, P], F32R, tag="psv", name="ps_v")
                        nc.tensor.transpose(
                            ps_v[:], vt_sb[:, j * P:(j + 1) * P], ident[:])
                        nc.vector.tensor_copy(
                            v_s[:, i * (ACH // P) + j, :], ps_v[:])
                    return emit

                def mk_rope(i, b):
                    def emit():
                        t0 = i * ACH
                        q_ap = qkt_s[0:RD, b, t0:t0 + ACH]
                        ps_r = psR.tile([RD, ACH], F32, tag="psr", name="ps_r")
                        nc.tensor.matmul(ps_r[:], _r32(srot_s[:]), _r32(q_ap),
                                         start=True, stop=True)
                        t1 = ropep.tile([RD, ACH], F32, tag="t1")
                        nc.gpsimd.tensor_mul(t1[:], q_ap,
                                             cos_s[:, t0:t0 + ACH])
                        t2 = ropep.tile([RD, ACH], F32, tag="t2")
                        nc.vector.tensor_mul(t2[:], ps_r[:],
                                             sin_s[:, t0:t0 + ACH])
                        nc.gpsimd.tensor_add(q_ap, t1[:], t2[:])
                    return emit

                vrope = []
                for i in range(NACH):
                    t0 = i * ACH
                    if i == 0:
                        xnt_i = xnt_first
                    else:
                        xnt_i = xpool.tile([P, HCH, ACH], F32R, tag="xnt")
                        for hc in range(HCH):
                            nc.sync.dma_start(xnt_i[:, hc, :],
                                              x_r[hc][:, t0:t0 + ACH])
                    # 4 col-blocks: q0, q1, k, v
                    for cb in range(4):
                        ps_c = psA.tile([P, ACH], F32, tag="psc")
                        for hc in range(HCH):
                            nc.tensor.matmul(
                                ps_c[:], _r32(w_s[:, hc, cb * P:(cb + 1) * P]),
                                _r32(xnt_i[:, hc, :]),
                                start=(hc == 0), stop=(hc == HCH - 1))
                        if cb < 3:
                            nc.vector.tensor_copy(qkt_s[:, cb, t0:t0 + ACH],
                                                  ps_c[:])
                        else:
                            vt_sb = vtp.tile([P, ACH], F32R, tag=f"vt{i}",
                                             name=f"vt_sb{i}")
                            nc.vector.tensor_copy(vt_sb[:], ps_c[:])
                            vrope += [mk_vtr(i, vt_sb, j)
                                      for j in range(ACH // P)]
                            vrope += [mk_rope(i, b) for b in range(3)]
                        for _ in range(2):
                            if vrope:
                                vrope.pop(0)()
                # stage Wo during phase B (first needed by phase C of group 0)
                for h in range(NHC):
                    nc.sync.dma_start(wo_s[:, h, :], wo_r[h])
                while vrope:
                    vrope.pop(0)()

            # ---------------- phase B + C fused ----------------
            # One psum tag ("ps") is shared by scores, Wo-product, and the
            # reciprocal broadcast so all four buffers stay busy.  Each query
            # group's Wo matmuls ("phase C") are injected into the NEXT
            # group's score/PV step loop so the PE always has independent
            # work while the scalar engine chews through exp.
            with (
                tc.tile_pool(name="psX", bufs=4, space="PSUM") as psX,
                tc.tile_pool(name="psO", bufs=1, space="PSUM") as psO,
                tc.tile_pool(name="psD", bufs=1, space="PSUM") as psD,
                tc.tile_pool(name="ptp", bufs=LAG + 2) as ptp,
                tc.tile_pool(name="den", bufs=2) as denp,
                tc.tile_pool(name="outp", bufs=4) as outp,
            ):
                c_work = []   # pending phase-C emitters from the previous qg

                def emit_c(ti, nt):
                    ps_p = psX.tile([P, 512], F32, tag="ps", name="ps_p")
                    for h in range(NHC):
                        nc.tensor.matmul(
                            ps_p[:],
                            _r32(ot_s[:, h, ti * P:(ti + 1) * P]),
                            _r32(wo_s[:, h, nt * 512:(nt + 1) * 512]),
                            start=(h == 0), stop=(h == NHC - 1))
                    out_sb = outp.tile([P, 512], F32, tag="osb")
                    nc.vector.tensor_copy(out_sb[:], ps_p[:])
                    nc.sync.dma_start(
                        pt_out[ti][:, nt * 512:(nt + 1) * 512], out_sb[:])

                for qg in range(NQG):
                    q0 = qg * QG
                    nkt = (QG // P) * (qg + 1)
                    ps_o = [psO.tile([P, QG], F32, tag=f"pso{h}",
                                     name=f"ps_o{h}")
                            for h in range(NHC)]
                    ps_d = [psD.tile([1, QG], F32, tag=f"psd{h}",
                                     name=f"ps_d{h}")
                            for h in range(NHC)]
                    q_rhs = [_r32(qkt_s[:, h, q0:q0 + QG]) for h in range(NHC)]

                    steps = [(kt, h) for kt in range(nkt) for h in range(NHC)]
                    # interleave leftover phase-C tiles ~evenly across steps
                    cper = -(-len(c_work) // len(steps))
                    pend = []

                    def emit_front(kt, h):
                        # diagonal block: columns < d*128 are fully masked —
                        # skip them in scores/exp/PV/den entirely
                        d = kt - (QG // P) * qg
                        c0 = d * P if d > 0 else 0
                        ps_s = psX.tile([P, QG], F32, tag="ps", name="ps_s")
                        nc.tensor.matmul(
                            ps_s[:, c0:], _r32(qkt_s[:, 2, kt * P:(kt + 1) * P]),
                            q_rhs[h][:, c0:], start=True, stop=True)
                        p_t = ptp.tile([P, QG], F32R, tag="pt")
                        nc.scalar.activation(
                            p_t[:, c0:], ps_s[:, c0:],
                            mybir.ActivationFunctionType.Exp, scale=SCALE)
                        if d >= 0:  # triangular mask on the 128-wide diagonal
                            nc.gpsimd.tensor_mul(
                                p_t[:, d * P:(d + 1) * P],
                                p_t[:, d * P:(d + 1) * P], mask_s[:])
                        return c0, p_t

                    def emit_back(kt, h, c0, p_t):
                        nc.tensor.matmul(ps_o[h][:, c0:],
                                         _r32(v_s[:, kt, :]),
                                         _r32(p_t[:, c0:]),
                                         start=(kt == 0),
                                         stop=(kt == nkt - 1))
                        nc.tensor.matmul(ps_d[h][:, c0:],
                                         _r32(ones_col[:]), _r32(p_t[:, c0:]),
                                         start=(kt == 0),
                                         stop=(kt == nkt - 1))

                    for si, (kt, h) in enumerate(steps):
                        pend.append((kt, h, *emit_front(kt, h)))
                        for _ in range(cper):
                            if c_work:
                                emit_c(*c_work.pop(0))
                        if len(pend) > LAG:
                            emit_back(*pend.pop(0))
                    for item in pend:
                        emit_back(*item)
                    while c_work:
                        emit_c(*c_work.pop(0))

                    for h in range(NHC):
                        den = denp.tile([1, QG], F32, tag="den")
                        nc.vector.tensor_scalar(
                            den[:], ps_d[h][:],
                            sinke_s[:, h:h + 1], None,
                            mybir.AluOpType.add)
                        rec = denp.tile([1, QG], F32R, tag="rec")
                        with nc.allow_low_precision(reason="f32r for PE bcast"):
                            nc.vector.reciprocal(rec[:], den[:])
                        ps_bc = psX.tile([P, QG], F32, tag="ps", name="ps_bc")
                        nc.tensor.matmul(ps_bc[:], _r32(ones_row[:]),
                                         _r32(rec[:]),
                                         start=True, stop=True)
                        rec_bc = denp.tile([P, QG], F32, tag="recbc")
                        nc.scalar.activation(rec_bc[:], ps_bc[:],
                                             mybir.ActivationFunctionType.Copy)
                        nc.vector.tensor_mul(ot_s[:, h, q0:q0 + QG],
                                             ps_o[h][:], rec_bc[:])

                    c_work = [(qg * (QG // P) + tl, nt)
                              for tl in range(QG // P)
                              for nt in range(H // 512)]

                # drain the last query group's phase C
                while c_work:
                    emit_c(*c_work.pop(0))

    nc.finalize()
    return nc


# --------------------------------------------------------------------------
# Launch 2: MoE expert FF (2 experts per core, capacity c_cap tokens each)
# --------------------------------------------------------------------------

def _n_chunks(c):
    """Split c (multiple of 128) into moving-dim chunks of <=512 (bf16)."""
    out = [512] * (c // 512)
    if c % 512:
        out.append(c % 512)
    assert sum(out) == c and all(x % P == 0 for x in out)
    return out


def build_moe(c_cap):
    nc = _mk_nc()
    xgt = nc.dram_tensor("xgt", [EPC, H, c_cap], BF16, kind="ExternalInput")
    # combine weights in token-partition layout: [EPC * c_cap/P, P]
    wcol = nc.dram_tensor("wcol", [EPC * (c_cap // P), P], F32,
                          kind="ExternalInput")
    weg = nc.dram_tensor("weg", [EPC, H, FF], BF16, kind="ExternalInput")
    weu = nc.dram_tensor("weu", [EPC, H, FF], BF16, kind="ExternalInput")
    wed = nc.dram_tensor("wed", [EPC, FF, H], BF16, kind="ExternalInput")
    contrib = nc.dram_tensor("contrib", [EPC * c_cap, H], BF16,
                             kind="ExternalOutput")
    co = contrib.rearrange("(ec tc p) h -> ec tc p h", p=P, ec=EPC)
    wc_r = wcol.rearrange("(ec tc) p -> ec tc p", ec=EPC)
    xg_r = [xgt[e].rearrange("(hc p) c -> hc p c", p=P) for e in range(EPC)]
    wg_r = [weg[e].rearrange("(hc p) f -> hc p f", p=P) for e in range(EPC)]
    wu_r = [weu[e].rearrange("(hc p) f -> hc p f", p=P) for e in range(EPC)]

    nch = _n_chunks(c_cap)
    ffc_n = FF // P  # 4

    with tile.TileContext(nc) as tc:
        with (
            tc.tile_pool(name="xg", bufs=1) as xgp,
            tc.tile_pool(name="wgu", bufs=2) as wgup,
            tc.tile_pool(name="wd", bufs=2) as wdp,
            tc.tile_pool(name="hgu", bufs=2) as hgup,
            tc.tile_pool(name="act", bufs=3) as actp,
            tc.tile_pool(name="wr", bufs=2) as wrp,
            tc.tile_pool(name="outp", bufs=4) as outp,
            tc.tile_pool(name="psG", bufs=2, space="PSUM") as psG,
            tc.tile_pool(name="psU", bufs=2, space="PSUM") as psU,
            tc.tile_pool(name="psC", bufs=3, space="PSUM") as psC,
        ):
            # prefetch everything in compute order: all of expert 0's
            # inputs, then expert 1's (DMA total ~23MB < compute time)
            stage = {}
            for e in range(EPC):
                xg_s = xgp.tile([P, HCH, c_cap], BF16, tag=f"xg{e}",
                                name=f"xg{e}")
                wg_s = wgup.tile([P, HCH, FF], BF16, tag="wg", name=f"wg{e}")
                wu_s = wgup.tile([P, HCH, FF], BF16, tag="wu", name=f"wu{e}")
                for hc in range(HCH):
                    # expert 0: interleave per hc so compute starts at once
                    nc.sync.dma_start(xg_s[:, hc, :], xg_r[e][hc])
                    if e == 0:
                        nc.sync.dma_start(wg_s[:, hc, :], wg_r[e][hc])
                        nc.sync.dma_start(wu_s[:, hc, :], wu_r[e][hc])
                if e > 0:
                    for hc in range(HCH):
                        nc.sync.dma_start(wg_s[:, hc, :], wg_r[e][hc])
                        nc.sync.dma_start(wu_s[:, hc, :], wu_r[e][hc])
                wcol_s = wrp.tile([P, c_cap // P], F32, tag="wcol",
                                  name=f"wcol{e}")
                nc.sync.dma_start(wcol_s[:],
                                  wc_r[e].rearrange("tc p -> p tc"))
                wd_s = wdp.tile([P, ffc_n, H], BF16, tag="wd", name=f"wd{e}")
                nc.sync.dma_start(wd_s[:],
                                  wed[e].rearrange("(fc p) h -> p fc h", p=P))
                stage[e] = (xg_s, wg_s, wu_s, wcol_s, wd_s)

            for e in range(EPC):
                xg_s, wg_s, wu_s, wcol_s, wd_s = stage[e]
                nco = 0
                for nsz in nch:
                    hgu = hgup.tile([P, ffc_n, 512], BF16, tag="hgu")
                    for fc in range(ffc_n):
                        ps_g = psG.tile([P, 512], F32, tag="psg")
                        ps_u = psU.tile([P, 512], F32, tag="psu")
                        for hc in range(HCH):
                            nc.tensor.matmul(
                                ps_g[:, :nsz],
                                wg_s[:, hc, fc * P:(fc + 1) * P],
                                xg_s[:, hc, nco:nco + nsz],
                                start=(hc == 0), stop=(hc == HCH - 1))
                            nc.tensor.matmul(
                                ps_u[:, :nsz],
                                wu_s[:, hc, fc * P:(fc + 1) * P],
                                xg_s[:, hc, nco:nco + nsz],
                                start=(hc == 0), stop=(hc == HCH - 1))
                        sg = actp.tile([P, 512], F32, tag="sg")
                        nc.scalar.activation(sg[:, :nsz], ps_g[:, :nsz],
                                             mybir.ActivationFunctionType.Silu)
                        nc.vector.tensor_mul(hgu[:, fc, :nsz],
                                             sg[:, :nsz], ps_u[:, :nsz])
                    # down projection for this chunk's token tiles
                    for tl in range(nsz // P):
                        ti = nco // P + tl
                        for nt in range(H // 512):
                            ps_c = psC.tile([P, 512], F32, tag="psc")
                            for fc in range(ffc_n):
                                nc.tensor.matmul(
                                    ps_c[:],
                                    hgu[:, fc, tl * P:(tl + 1) * P],
                                    wd_s[:, fc, nt * 512:(nt + 1) * 512],
                                    start=(fc == 0), stop=(fc == ffc_n - 1))
                            out_sb = outp.tile([P, 512], BF16, tag="osb")
                            nc.scalar.activation(
                                out_sb[:], ps_c[:],
                                mybir.ActivationFunctionType.Copy,
                                scale=wcol_s[:, ti:ti + 1])
                            nc.sync.dma_start(
                                co[e, ti][:, nt * 512:(nt + 1) * 512],
                                out_sb[:])
                    nco += nsz

    nc.finalize()
    return nc


# --------------------------------------------------------------------------
# Host-side routing (numpy mirror of the reference MoE gate)
# --------------------------------------------------------------------------

def _routing(h1, ln2_w, gate_w, gate_bias):
    var = np.mean(h1 * h1, axis=-1, keepdims=True)
    xf = (ln2_w * (h1 / np.sqrt(var + EPS))).astype(np.float32)
    logits = xf @ gate_w.T
    s = 1.0 / (1.0 + np.exp(-logits))
    sfc = s + gate_bias[None]
    n = sfc.shape[0]
    gview = sfc.reshape(n, G, E // G)
    gsort = np.sort(gview, axis=-1)
    group_scores = gsort[..., -1] + gsort[..., -2]
    gidx = np.argsort(-group_scores, kind="stable", axis=-1)[:, :TG]
    gmask = np.zeros((n, G), np.bool_)
    np.put_along_axis(gmask, gidx, True, axis=1)
    smask = np.repeat(gmask, E // G, axis=1)
    tmp = np.where(smask, sfc, -np.inf)
    tidx = np.argsort(-tmp, kind="stable", axis=-1)[:, :TK]
    tw = np.take_along_axis(s, tidx, axis=1)
    tw = tw / (tw.sum(-1, keepdims=True) + 1e-20)
    tw = tw * ROUTE_SCALE
    cw = np.zeros((n, E), np.float32)
    np.put_along_axis(cw, tidx, tw.astype(np.float32), axis=1)
    return xf, cw


# --------------------------------------------------------------------------
# Entry point
# --------------------------------------------------------------------------

_NC_CACHE = {}


def _get_nc(key, builder, *args):
    if key not in _NC_CACHE:
        _NC_CACHE[key] = builder(*args)
    return _NC_CACHE[key]


def kernel(hidden_states, cos, sin, ln1_w, ln2_w, Wq, Wk, Wv, Wo,
           sink_bias, gate_w, gate_bias, Weg, Weu, Wed, _profile=None):
    hidden_states, cos, sin, ln1_w, ln2_w = map(
        np.asarray, (hidden_states, cos, sin, ln1_w, ln2_w))
    Wq, Wk, Wv, Wo, sink_bias = map(np.asarray, (Wq, Wk, Wv, Wo, sink_bias))
    gate_w, gate_bias, Weg, Weu, Wed = map(
        np.asarray, (gate_w, gate_bias, Weg, Weu, Wed))
    b, s, _ = hidden_states.shape
    x = np.ascontiguousarray(hidden_states.reshape(T, H), dtype=np.float32)

    # host: rmsnorm1 + transpose (pre-gate path stays fp32)
    var = np.mean(x * x, axis=-1, keepdims=True)
    xn = x * (1.0 / np.sqrt(var + EPS))
    xnt = np.ascontiguousarray(xn.T)

    # rope tables in transposed layout
    cosb = cos.reshape(T, RD).astype(np.float32)
    sinb = sin.reshape(T, RD).astype(np.float32)
    cost = np.ascontiguousarray(cosb.T)
    sint = np.ascontiguousarray(sinb.T)

    # signed half-swap: rot[j] = -q[j+RH] (j<RH), q[j-RH] (j>=RH)
    srot = np.zeros((RD, RD), np.float32)
    for j in range(RH):
        srot[j + RH, j] = -1.0
        srot[j, j + RH] = 1.0

    # fold ln1 into the QKV weights
    wq_f = (ln1_w[:, None] * Wq).astype(np.float32)
    wk_f = (ln1_w[:, None] * Wk).astype(np.float32)
    wv_f = (ln1_w[:, None] * Wv).astype(np.float32)

    # triangular causal mask for the 128-wide diagonal sub-block
    kp = np.arange(P)[:, None]
    qf = np.arange(P)[None, :]
    masks = (qf >= kp).astype(np.float32)

    in_maps = []
    for c in range(N_CORES):
        h0 = NHC * c
        g0 = h0 // (16 // 4)  # kv head
        w_cat = np.concatenate(
            [wq_f[:, h0 * HD:(h0 + NHC) * HD],
             wk_f[:, g0 * HD:(g0 + 1) * HD],
             wv_f[:, g0 * HD:(g0 + 1) * HD]], axis=1)
        in_maps.append({
            "xnt": xnt,
            "w_all": np.ascontiguousarray(w_cat),
            "wo": np.ascontiguousarray(Wo[h0 * HD:(h0 + NHC) * HD, :]),
            "cost": cost,
            "sint": sint,
            "srot": srot,
            "sinke": np.exp(sink_bias[h0:h0 + NHC]).reshape(1, NHC)
                       .astype(np.float32),
            "masks": masks,
        })

    nc1 = _get_nc("attn", build_attn)
    res1 = run_bass_kernel_spmd(nc1, in_maps, core_ids=list(range(N_CORES)),
                                trace=_profile is not None)
    h1 = x.copy()
    for c in range(N_CORES):
        h1 += res1.results[c]["partial"]

    xf, cw = _routing(h1, np.asarray(ln2_w), np.asarray(gate_w),
                      np.asarray(gate_bias))

    idxs = [np.nonzero(cw[:, e] > 0)[0] for e in range(E)]
    maxc = max(len(ix) for ix in idxs)
    c_cap = max(P, -(-maxc // P) * P)

    xf_t = xf.T  # [H, n]
    in_maps2 = []
    for c in range(N_CORES):
        xg = np.zeros((EPC, H, c_cap), BF16_NP)
        wr = np.zeros((EPC, c_cap // P, P), np.float32)
        for j in range(EPC):
            e = EPC * c + j
            ix = idxs[e]
            xg[j, :, :len(ix)] = xf_t[:, ix].astype(BF16_NP)
            wr[j].reshape(-1)[:len(ix)] = cw[ix, e]
        in_maps2.append({
            "xgt": xg,
            "wcol": wr.reshape(EPC * (c_cap // P), P),
            "weg": np.ascontiguousarray(Weg[EPC * c:EPC * (c + 1)]
                                        .astype(BF16_NP)),
            "weu": np.ascontiguousarray(Weu[EPC * c:EPC * (c + 1)]
                                        .astype(BF16_NP)),
            "wed": np.ascontiguousarray(Wed[EPC * c:EPC * (c + 1)]
                                        .astype(BF16_NP)),
        })

    nc2 = _get_nc(("moe", c_cap), build_moe, c_cap)
    res2 = run_bass_kernel_spmd(nc2, in_maps2, core_ids=list(range(N_CORES)),
                                trace=_profile is not None)

    out = h1
    for c in range(N_CORES):
        cb = res2.results[c]["contrib"].reshape(EPC, c_cap, H)
        for j in range(EPC):
            e = EPC * c + j
            ix = idxs[e]
            out[ix] += cb[j, :len(ix)].astype(np.float32)

    if _profile is not None:
        _profile["attn_ns"] = res1.exec_time_ns
        _profile["moe_ns"] = res2.exec_time_ns
        _profile["res1"] = res1
        _profile["res2"] = res2

    return out.reshape(b, s, H)
